# revision 20
# baseline (speedup 1.0000x reference)
"""Fused single-core Bass kernel for a tiny 2-block causal transformer.

Shapes (hardcoded): S=26 seq, D=7 embed, DA=11 attn, V=29 vocab.
Returns (out [26,29] f32, att [26,26] f32) matching the reference.

Strategy: the whole forward is latency-bound, so everything lives in one
SBUF const image (one input DMA), activations are kept transposed
([feature, seq]) so all matmuls chain on the TensorEngine with biases
folded in via appended ones-rows, the causal mask is applied by
preloading the attention PSUM banks with -1e30 and accumulating the
QK^T matmul on top, and softmax uses the ScalarEngine's fused
exp+row-sum. The 8 cores run the identical tiny program (SPMD
replication); core 0's output is returned.
"""

import numpy as np

S, D, DA, V = 26, 7, 11, 29
NEG = -1.0e30

# const image column layout (29 partitions x WCOL f32)
_C_EMB = 0        # emb_table [29, 7]
_C_OH = 7         # onehotT  [29, 26]
_C_QK0 = 33       # Wqk0 [8, 22]  rows0:7 = [wq0.T | wk0.T], row7 = [bq0 | bk0]
_C_V0 = 55        # Wv0  [8, 11]
_C_F0 = 66        # Wf0  [12, 7]  rows0:11 = wf0.T, row11 = bf0
_C_QK1 = 73
_C_V1 = 95
_C_F1 = 106
_C_OUT = 113      # Wout [8, 29]
_C_POS = 142      # posT [7, 26]
_C_MSK = 168      # causal mask bias [26, 26]: 0.0 where j<=i else -1e30
WCOL = 194

N_CORES = 8

_cache = {}


def _build():
    """Build + compile the Bass program once per process."""
    import concourse.bass as bass
    import concourse.bacc as bacc
    import concourse.mybir as mybir
    import concourse.tile as tile

    f32 = mybir.dt.float32
    nc = bacc.Bacc("TRN2", target_bir_lowering=False, debug=False)

    consts = nc.dram_tensor("consts", [29, WCOL], f32, kind="ExternalInput")
    out_d = nc.dram_tensor("out", [S, V], f32, kind="ExternalOutput")
    att_d = nc.dram_tensor("att", [S, S], f32, kind="ExternalOutput")

    with tile.TileContext(nc) as tc:
        with (
            tc.tile_pool(name="sb", bufs=1) as sb,
            tc.tile_pool(name="pmm", bufs=4, space=bass.MemorySpace.PSUM) as pmm,
            tc.tile_pool(name="patt", bufs=1, space=bass.MemorySpace.PSUM) as patt,
        ):
            C = sb.tile([29, WCOL], f32, tag="C")
            nc.sync.dma_start(C[:, :], consts[:, :])

            # activation tiles with a trailing ones-row for folded biases
            embA = sb.tile([8, S], f32, tag="embA")    # embT rows0:7, row7 = 1
            h1A = sb.tile([8, S], f32, tag="h1A")
            h2A = sb.tile([8, S], f32, tag="h2A")
            resA = [sb.tile([12, S], f32, name=f"resA{b}", tag=f"resA{b}")
                    for b in range(2)]
            # whole-tile memset to 1.0 (memset must start at partition 0);
            # the data rows are overwritten later, leaving the ones-row.
            for t in (embA, h1A, h2A, resA[0], resA[1]):
                nc.vector.memset(t[:, :], 1.0)

            # embedding: embT = emb_table.T @ onehot  (+ posT)
            embp = pmm.tile([D, S], f32, tag="mm")
            nc.tensor.matmul(embp[:, :], C[0:29, _C_EMB:_C_EMB + D],
                             C[0:29, _C_OH:_C_OH + S])
            nc.vector.tensor_add(embA[0:D, :], embp[:, :], C[0:D, _C_POS:_C_POS + S])

            def block(b, hA, qk_off, v_off, f_off, hA_next):
                # qT and kT: [11, 26] each (separate matmuls; compute-engine
                # PSUM reads must start at a quadrant-aligned partition)
                qp = pmm.tile([DA, S], f32, name=f"qp{b}", tag="mm")
                kp = pmm.tile([DA, S], f32, name=f"kp{b}", tag="mm")
                nc.tensor.matmul(qp[:, :], C[0:8, qk_off:qk_off + DA], hA[:, :])
                nc.tensor.matmul(kp[:, :], C[0:8, qk_off + DA:qk_off + 2 * DA],
                                 hA[:, :])
                qs = sb.tile([DA, S], f32, name=f"qs{b}", tag=f"qs{b}")
                ks = sb.tile([DA, S], f32, name=f"ks{b}", tag=f"ks{b}")
                nc.scalar.copy(qs[:, :], qp[:, :])
                nc.vector.tensor_copy(ks[:, :], kp[:, :])

                # v natural: [26, 11]
                vp = pmm.tile([S, DA], f32, tag="mm")
                nc.tensor.matmul(vp[:, :], hA[:, :], C[0:8, v_off:v_off + DA])
                vs = sb.tile([S, DA], f32, tag=f"vs{b}")
                nc.vector.tensor_copy(vs[:, :], vp[:, :])

                # att = q @ k.T
                attp = patt.tile([S, S], f32, name=f"attp{b}", tag=f"attp{b}")
                nc.tensor.matmul(attp[:, :], qs[:, :], ks[:, :])

                # causal mask as additive -1e30 bias, then fused exp+row-sum
                # (raw scores are small, so exp without max-sub is safe;
                # masked entries underflow to exactly 0).
                am = sb.tile([S, S], f32, name=f"am{b}", tag=f"am{b}")
                nc.vector.tensor_add(am[:, :], attp[:, :],
                                     C[0:S, _C_MSK:_C_MSK + S])
                es = sb.tile([S, S], f32, name=f"es{b}", tag=f"es{b}")
                ssum = sb.tile([S, 1], f32, name=f"ssum{b}", tag=f"ssum{b}")
                nc.scalar.activation(es[:, :], am[:, :],
                                     mybir.ActivationFunctionType.Exp,
                                     accum_out=ssum[:, :])
                rr = sb.tile([S, 1], f32, name=f"rr{b}", tag=f"rr{b}")
                nc.vector.reciprocal(rr[:, :], ssum[:, :])
                atts = sb.tile([32, 32], f32, name=f"atts{b}", tag=f"atts{b}")
                nc.gpsimd.memset(atts[:, :], 0.0)
                nc.vector.tensor_scalar_mul(atts[0:S, 0:S], es[:, :], rr[:, :])

                # att.T via DVE 32x32 stream transpose, then res.T = v.T @ att.T
                attTs = sb.tile([32, 32], f32, tag=f"attTs{b}")
                nc.vector.transpose(attTs[:, :], atts[:, :])
                resp = pmm.tile([DA, S], f32, tag="mm")
                nc.tensor.matmul(resp[:, :], vs[:, :], attTs[0:S, 0:S])
                nc.scalar.copy(resA[b][0:DA, :], resp[:, :])

                # block out: hT = wf.T-aug @ resA
                hp = pmm.tile([D, S], f32, tag="mm")
                nc.tensor.matmul(hp[:, :], C[0:12, f_off:f_off + D], resA[b][:, :])
                nc.scalar.copy(hA_next[0:D, :], hp[:, :])
                return atts

            block(0, embA, _C_QK0, _C_V0, _C_F0, h1A)
            att1 = block(1, h1A, _C_QK1, _C_V1, _C_F1, h2A)
            nc.sync.dma_start(att_d[:, :], att1[0:S, 0:S])

            outp = pmm.tile([S, V], f32, tag="mm")
            nc.tensor.matmul(outp[:, :], h2A[:, :], C[0:8, _C_OUT:_C_OUT + V])
            outs = sb.tile([S, V], f32, tag="outs")
            nc.scalar.copy(outs[:, :], outp[:, :])
            nc.sync.dma_start(out_d[:, :], outs[:, :])

    nc.compile()
    return nc


def _get_nc():
    if "nc" not in _cache:
        _cache["nc"] = _build()
    return _cache["nc"]


def pack_consts(inputs):
    x = np.asarray(inputs["x"]).astype(np.int64)
    C = np.zeros((29, WCOL), np.float32)
    C[:, _C_EMB:_C_EMB + D] = np.asarray(inputs["emb_table"], np.float32)
    C[x, _C_OH + np.arange(S)] = 1.0
    for off, (wq, bq, wk, bk) in (
        (_C_QK0, ("w_q0", "b_q0", "w_k0", "b_k0")),
        (_C_QK1, ("w_q1", "b_q1", "w_k1", "b_k1")),
    ):
        C[0:D, off:off + DA] = np.asarray(inputs[wq], np.float32).T
        C[D, off:off + DA] = np.asarray(inputs[bq], np.float32)
        C[0:D, off + DA:off + 2 * DA] = np.asarray(inputs[wk], np.float32).T
        C[D, off + DA:off + 2 * DA] = np.asarray(inputs[bk], np.float32)
    for off, (wv, bv) in ((_C_V0, ("w_v0", "b_v0")), (_C_V1, ("w_v1", "b_v1"))):
        C[0:D, off:off + DA] = np.asarray(inputs[wv], np.float32).T
        C[D, off:off + DA] = np.asarray(inputs[bv], np.float32)
    for off, (wf, bf) in ((_C_F0, ("w_f0", "b_f0")), (_C_F1, ("w_f1", "b_f1"))):
        C[0:DA, off:off + D] = np.asarray(inputs[wf], np.float32).T
        C[DA, off:off + D] = np.asarray(inputs[bf], np.float32)
    C[0:D, _C_OUT:_C_OUT + V] = np.asarray(inputs["w_out"], np.float32).T
    C[D, _C_OUT:_C_OUT + V] = np.asarray(inputs["b_out"], np.float32)
    C[0:D, _C_POS:_C_POS + S] = np.asarray(inputs["pos"], np.float32)[:S].T
    C[0:S, _C_MSK:_C_MSK + S] = np.where(
        np.tril(np.ones((S, S), bool)), 0.0, NEG).astype(np.float32)
    return C


def run(inputs, trace=False):
    """Run on HW; returns (results_dict_core0, BassKernelResults)."""
    from concourse.bass_utils import run_bass_kernel_spmd

    nc = _get_nc()
    in_map = {"consts": pack_consts(inputs)}
    res = run_bass_kernel_spmd(
        nc, [dict(in_map) for _ in range(N_CORES)], list(range(N_CORES)),
        trace=trace,
    )
    return res.results[0], res


def kernel(**inputs):
    r0, _ = run(inputs, trace=False)
    return r0["out"].astype(np.float32), r0["att"].astype(np.float32)


# revision 36
# speedup vs baseline: 1.1969x; 1.1969x over previous
"""Fused single-core raw-Bass kernel for a tiny 2-block causal transformer.

Shapes (hardcoded): S=26 seq, D=7 embed, DA=11 attn, V=29 vocab.
kernel(**inputs) takes the full unsharded inputs (as in
reference.setup_inputs()) and returns (out [26,29] f32, att [26,26] f32).

Design (latency-bound problem):
- all weights/constants packed on host into one [29, 236] f32 image;
  two DMAs bring it in (the half the first matmuls need goes first);
- activations kept transposed ([feature, seq]) so all matmuls chain
  directly on the TensorEngine, with biases folded in via ones-rows;
- q and k are one matmul: k's weight columns sit at lhsT free offset
  32 so the PSUM copy-out of k starts at a quadrant-aligned partition;
- causal mask applied by preloading the attention PSUM bank with -1e30
  and accumulating the QK^T matmul on top (raw scores are small, so
  exp without max-subtraction is safe; masked entries underflow to 0);
- softmax: exp on ScalarE, row-sum/reciprocal/scale + 32x32 stream
  transpose on VectorE;
- hand-scheduled raw Bass (no Tile): explicit per-engine streams,
  monotonic semaphores, manual PSUM bank reuse, no explicit teardown
  (the NEFF epilogue resets every semaphore anyway).

The 8 cores run the identical program (SPMD replication, per the
sharding hint); core 0's output is returned.
"""

import numpy as np

S, D, DA, V = 26, 7, 11, 29
NEG = -1.0e30

# const image column layout (29 partitions x WCOL f32).
# hot half 0:113 = what the first matmuls need; cold half 113:236.
_C_EMB = 0        # emb_table [29, 7]
_C_OH = 7         # onehotT  [29, 26]
_C_QK0 = 33       # Wqk0 [8, 43]: wq.T at +0:11, wk.T at +32:43, biases row 7
_C_V0 = 76        # Wv0  [8, 11]
_C_POS = 87       # posT [7, 26]
_C_HOT = 113
_C_F0 = 113       # Wf0  [12, 7]: wf.T rows 0:11, bias row 11
_C_QK1 = 120
_C_V1 = 163
_C_F1 = 174
_C_OUT = 181      # Wout [8, 29]
_C_MSK = 210      # causal mask bias [26, 26]: 0.0 where j<=i else -1e30
_C_ID = 236       # identity [26, 26] (PE mask-preload: attp = id.T @ mask)
WCOL = 262
_KOFF = 32        # k's offset inside a Wqk block

N_CORES = 8

_cache = {}


def build(wait_out=True):
    import concourse.bacc as bacc
    import concourse.mybir as mybir

    f32 = mybir.dt.float32
    EXP = mybir.ActivationFunctionType.Exp
    X = mybir.AxisListType.X
    nc = bacc.Bacc("TRN2", target_bir_lowering=False, debug=False)

    consts = nc.dram_tensor("consts", [29, WCOL], f32, kind="ExternalInput")
    out_d = nc.dram_tensor("out", [S, V], f32, kind="ExternalOutput")
    att_d = nc.dram_tensor("att", [S, S], f32, kind="ExternalOutput")

    def sb(name, shape):
        return nc.alloc_sbuf_tensor(name, list(shape), f32)

    C = sb("C", [29, WCOL])
    embA, h1A, h2A = sb("embA", [8, S]), sb("h1A", [8, S]), sb("h2A", [8, S])
    resA = [sb(f"resA{b}", [12, S]) for b in range(2)]
    qs = [sb(f"qs{b}", [DA, S]) for b in range(2)]
    ks = [sb(f"ks{b}", [DA, S]) for b in range(2)]
    vs = [sb(f"vs{b}", [S, DA]) for b in range(2)]
    es = [sb(f"es{b}", [S, S]) for b in range(2)]
    ssum = [sb(f"ssum{b}", [S, 1]) for b in range(2)]
    rr = [sb(f"rr{b}", [S, 1]) for b in range(2)]
    atts = [sb(f"atts{b}", [32, 32]) for b in range(2)]
    attT = [sb(f"attT{b}", [32, 32]) for b in range(2)]
    outs = sb("outs", [S, V])
    zb = sb("zb", [S, 1])

    # one PSUM bank per concurrently-live matmul output, reused over time
    B = [nc.alloc_psum_tensor(f"B{i}", [128, 128], f32) for i in range(6)]
    embp = B[0][0:D, 0:S]
    resp = [B[0][0:DA, 0:S], B[0][0:DA, 0:S]]
    qkp = [B[1][0:DA + _KOFF, 0:S], B[1][0:DA + _KOFF, 0:S]]
    vp = [B[2][0:S, 0:DA], B[2][0:S, 0:DA]]
    attp = [B[3][0:S, 0:S], B[4][0:S, 0:S]]
    hp = [B[5][0:D, 0:S], B[5][0:D, 0:S]]
    outp = B[5][0:S, 0:V]

    sD = nc.alloc_semaphore("sD")   # hot-half input DMA
    sE = nc.alloc_semaphore("sE")   # cold-half input DMA
    sP = nc.alloc_semaphore("sP")
    sA = nc.alloc_semaphore("sA")
    sV = nc.alloc_semaphore("sV")
    sO = nc.alloc_semaphore("sO")

    mask = C[0:S, _C_MSK:_C_MSK + S]

    # ---- SYNC: input DMAs first, output DMAs gated on DVE progress ----
    nc.sync.dma_start(C[:, 0:_C_HOT], consts[:, 0:_C_HOT]).then_inc(sD, 16)
    nc.sync.dma_start(C[:, _C_HOT:WCOL], consts[:, _C_HOT:WCOL]).then_inc(sE, 16)
    nc.sync.wait_ge(sV, 22)
    nc.sync.dma_start(att_d[:, :], atts[1][0:S, 0:S]).then_inc(sO, 16)
    nc.sync.wait_ge(sV, 26)
    nc.sync.dma_start(out_d[:, :], outs[:, :]).then_inc(sO, 16)
    if wait_out:
        nc.sync.wait_ge(sO, 32)

    # ---- ACT: q copies and exps  (A1..A4) ----
    nc.scalar.wait_ge(sP, 2)
    nc.scalar.copy(qs[0][:, :], qkp[0][0:DA, :]).then_inc(sA)       # A1
    nc.scalar.wait_ge(sP, 5)
    nc.scalar.wait_ge(sV, 1)
    nc.scalar.activation(es[0][:, :], attp[0], EXP,
                         bias=zb[:, :]).then_inc(sA)                # A2
    nc.scalar.wait_ge(sP, 8)
    nc.scalar.copy(qs[1][:, :], qkp[1][0:DA, :]).then_inc(sA)       # A3
    nc.scalar.wait_ge(sP, 11)
    nc.scalar.activation(es[1][:, :], attp[1], EXP,
                         bias=zb[:, :]).then_inc(sA)                # A4

    # ---- DVE: memsets, adds, copies, softmax tail  (V1..V26) ----
    # engine writes are posted, so same-engine data deps also get sem edges
    nc.vector.memset(zb[:, :], 0.0).then_inc(sV)                    # V1
    for t, val in ((embA, 1.0), (h1A, 1.0), (resA[0], 1.0), (atts[0], 0.0),
                   (h2A, 1.0), (resA[1], 1.0), (atts[1], 0.0)):
        nc.vector.memset(t[:, :], val).then_inc(sV)                 # V2..V8
    nc.vector.wait_ge(sP, 1)
    nc.vector.wait_ge(sV, 2)
    nc.vector.tensor_add(embA[0:D, :], embp,
                         C[0:D, _C_POS:_C_POS + S]).then_inc(sV)    # V9
    for b, (pqk, pv, pres, ph, hnxt) in enumerate(
            ((2, 4, 6, 7, h1A), (8, 10, 12, 13, h2A))):
        v0 = 8 * b  # V10..V17 for b=0, V18..V25 for b=1
        nc.vector.wait_ge(sP, pqk)
        nc.vector.tensor_copy(ks[b][:, :],
                              qkp[b][_KOFF:_KOFF + DA, :]).then_inc(sV)
        nc.vector.wait_ge(sP, pv)
        nc.vector.tensor_copy(vs[b][:, :], vp[b]).then_inc(sV)      # V11/ V19
        nc.vector.wait_ge(sA, 2 if b == 0 else 4)
        nc.vector.reduce_sum(ssum[b][:, :], es[b][:, :],
                             axis=X).then_inc(sV)                   # V12/ V20
        nc.vector.wait_ge(sV, v0 + 12)
        nc.vector.reciprocal(rr[b][:, :], ssum[b][:, :]).then_inc(sV)
        nc.vector.wait_ge(sV, v0 + 13)
        nc.vector.tensor_scalar_mul(atts[b][0:S, 0:S], es[b][:, :],
                                    rr[b][:, :]).then_inc(sV)       # V14/ V22
        nc.vector.wait_ge(sV, v0 + 14)
        nc.vector.transpose(attT[b][:, :], atts[b][:, :]).then_inc(sV)
        nc.vector.wait_ge(sP, pres)
        nc.vector.tensor_copy(resA[b][0:DA, :], resp[b]).then_inc(sV)
        nc.vector.wait_ge(sP, ph)
        nc.vector.tensor_copy(hnxt[0:D, :], hp[b]).then_inc(sV)     # V17/ V25
    nc.vector.wait_ge(sP, 14)
    nc.vector.tensor_copy(outs[:, :], outp).then_inc(sV)            # V26

    # ---- PE: the matmul chain  (P1..P14) ----
    # mask preloads are PE matmuls (id.T @ mask, start=True) so PSUM
    # has_written is set deterministically; an engine-write preload would
    # be OVERWRITTEN (not accumulated) by the QK^T matmul on a fresh bank.
    mm = nc.tensor.matmul
    idm = C[0:S, _C_ID:_C_ID + S]
    nc.tensor.wait_ge(sD, 16)
    mm(embp, C[0:29, _C_EMB:_C_EMB + D],
       C[0:29, _C_OH:_C_OH + S]).then_inc(sP)                       # P1
    for b, (qkoff, voff, foff, hA, vw, aw, kw) in enumerate(
            ((_C_QK0, _C_V0, _C_F0, embA, 9, 1, 10),
             (_C_QK1, _C_V1, _C_F1, h1A, 17, 3, 18))):
        nc.tensor.wait_ge(sV, vw)
        mm(qkp[b], C[0:8, qkoff:qkoff + _KOFF + DA],
           hA[:, :]).then_inc(sP)                                   # P2 / P9
        if b == 0:
            nc.tensor.wait_ge(sE, 16)
        mm(attp[b], idm, mask).then_inc(sP)                         # P3 / P10
        mm(vp[b], hA[:, :], C[0:8, voff:voff + DA]).then_inc(sP)    # P4 / P11
        nc.tensor.wait_ge(sA, aw)
        nc.tensor.wait_ge(sV, kw)
        mm(attp[b], qs[b][:, :], ks[b][:, :],
           start=False, stop=True, skip_group_check=True).then_inc(sP)
        nc.tensor.wait_ge(sV, 15 if b == 0 else 23)
        mm(resp[b], vs[b][:, :], attT[b][0:S, 0:S]).then_inc(sP)    # P6 / P13
        nc.tensor.wait_ge(sV, 16 if b == 0 else 24)
        mm(hp[b], C[0:12, foff:foff + D], resA[b][:, :]).then_inc(sP)
    nc.tensor.wait_ge(sV, 25)
    mm(outp, h2A[:, :], C[0:8, _C_OUT:_C_OUT + V]).then_inc(sP)     # P14

    nc.compile()
    return nc


def _get_nc():
    if "nc" not in _cache:
        _cache["nc"] = build(wait_out=False)
    return _cache["nc"]


def pack_consts(inputs):
    x = np.asarray(inputs["x"]).astype(np.int64)
    C = np.zeros((29, WCOL), np.float32)
    C[:, _C_EMB:_C_EMB + D] = np.asarray(inputs["emb_table"], np.float32)
    C[x, _C_OH + np.arange(S)] = 1.0
    for off, (wq, bq, wk, bk) in (
        (_C_QK0, ("w_q0", "b_q0", "w_k0", "b_k0")),
        (_C_QK1, ("w_q1", "b_q1", "w_k1", "b_k1")),
    ):
        C[0:D, off:off + DA] = np.asarray(inputs[wq], np.float32).T
        C[D, off:off + DA] = np.asarray(inputs[bq], np.float32)
        C[0:D, off + _KOFF:off + _KOFF + DA] = np.asarray(inputs[wk], np.float32).T
        C[D, off + _KOFF:off + _KOFF + DA] = np.asarray(inputs[bk], np.float32)
    for off, (wv, bv) in ((_C_V0, ("w_v0", "b_v0")), (_C_V1, ("w_v1", "b_v1"))):
        C[0:D, off:off + DA] = np.asarray(inputs[wv], np.float32).T
        C[D, off:off + DA] = np.asarray(inputs[bv], np.float32)
    for off, (wf, bf) in ((_C_F0, ("w_f0", "b_f0")), (_C_F1, ("w_f1", "b_f1"))):
        C[0:DA, off:off + D] = np.asarray(inputs[wf], np.float32).T
        C[DA, off:off + D] = np.asarray(inputs[bf], np.float32)
    C[0:D, _C_OUT:_C_OUT + V] = np.asarray(inputs["w_out"], np.float32).T
    C[D, _C_OUT:_C_OUT + V] = np.asarray(inputs["b_out"], np.float32)
    C[0:D, _C_POS:_C_POS + S] = np.asarray(inputs["pos"], np.float32)[:S].T
    C[0:S, _C_MSK:_C_MSK + S] = np.where(
        np.tril(np.ones((S, S), bool)), 0.0, NEG).astype(np.float32)
    C[0:S, _C_ID:_C_ID + S] = np.eye(S, dtype=np.float32)
    return C


def run(inputs, trace=False):
    """Run on HW; returns (results_dict_core0, BassKernelResults)."""
    from concourse.bass_utils import run_bass_kernel_spmd

    nc = _get_nc()
    in_map = {"consts": pack_consts(inputs)}
    res = run_bass_kernel_spmd(
        nc, [dict(in_map) for _ in range(N_CORES)], list(range(N_CORES)),
        trace=trace,
    )
    return res.results[0], res


def kernel(**inputs):
    r0, _ = run(inputs, trace=False)
    return r0["out"].astype(np.float32), r0["att"].astype(np.float32)


# revision 39
# speedup vs baseline: 1.2173x; 1.0171x over previous
"""Fused single-core raw-Bass kernel for a tiny 2-block causal transformer.

Shapes (hardcoded): S=26 seq, D=7 embed, DA=11 attn, V=29 vocab.
kernel(**inputs) takes the full unsharded inputs (as in
reference.setup_inputs()) and returns (out [26,29] f32, att [26,26] f32).

Design (latency-bound problem):
- all weights/constants packed on host into one [29, 236] f32 image;
  two DMAs bring it in (the half the first matmuls need goes first);
- activations kept transposed ([feature, seq]) so all matmuls chain
  directly on the TensorEngine, with biases folded in via ones-rows;
- q and k are one matmul: k's weight columns sit at lhsT free offset
  32 so the PSUM copy-out of k starts at a quadrant-aligned partition;
- causal mask applied by preloading the attention PSUM bank with -1e30
  and accumulating the QK^T matmul on top (raw scores are small, so
  exp without max-subtraction is safe; masked entries underflow to 0);
- softmax: exp on ScalarE, row-sum/reciprocal/scale + 32x32 stream
  transpose on VectorE;
- hand-scheduled raw Bass (no Tile): explicit per-engine streams,
  monotonic semaphores, manual PSUM bank reuse, no explicit teardown
  (the NEFF epilogue resets every semaphore anyway).

The 8 cores run the identical program (SPMD replication, per the
sharding hint); core 0's output is returned.
"""

import numpy as np

S, D, DA, V = 26, 7, 11, 29
NEG = -1.0e30

# const image column layout (29 partitions x WCOL f32).
# hot half 0:113 = what the first matmuls need; cold half 113:236.
_C_EMB = 0        # emb_table [29, 7]
_C_OH = 7         # onehotT  [29, 26]
_C_QK0 = 33       # Wqk0 [8, 43]: wq.T at +0:11, wk.T at +32:43, biases row 7
_C_V0 = 76        # Wv0  [8, 11]
_C_POS = 87       # posT [7, 26]
_C_HOT = 113
_C_F0 = 113       # Wf0  [12, 7]: wf.T rows 0:11, bias row 11
_C_QK1 = 120
_C_V1 = 163
_C_F1 = 174
_C_OUT = 181      # Wout [8, 29]
_C_MSK = 210      # causal mask bias [26, 26]: 0.0 where j<=i else -1e30
_C_ID = 236       # identity [26, 26] (PE mask-preload: attp = id.T @ mask)
WCOL = 262
_KOFF = 32        # k's offset inside a Wqk block

N_CORES = 8

_cache = {}


def build(wait_out=True):
    import concourse.bacc as bacc
    import concourse.mybir as mybir

    f32 = mybir.dt.float32
    EXP = mybir.ActivationFunctionType.Exp
    X = mybir.AxisListType.X
    nc = bacc.Bacc("TRN2", target_bir_lowering=False, debug=False)

    consts = nc.dram_tensor("consts", [29, WCOL], f32, kind="ExternalInput")
    out_d = nc.dram_tensor("out", [S, V], f32, kind="ExternalOutput")
    att_d = nc.dram_tensor("att", [S, S], f32, kind="ExternalOutput")

    def sb(name, shape):
        return nc.alloc_sbuf_tensor(name, list(shape), f32)

    C = sb("C", [29, WCOL])
    embA, h1A, h2A = sb("embA", [8, S]), sb("h1A", [8, S]), sb("h2A", [8, S])
    resA = [sb(f"resA{b}", [12, S]) for b in range(2)]
    qs = [sb(f"qs{b}", [DA, S]) for b in range(2)]
    ks = [sb(f"ks{b}", [DA, S]) for b in range(2)]
    vs = [sb(f"vs{b}", [S, DA]) for b in range(2)]
    es = [sb(f"es{b}", [S, S]) for b in range(2)]
    ssum = [sb(f"ssum{b}", [S, 1]) for b in range(2)]
    rr = [sb(f"rr{b}", [S, 1]) for b in range(2)]
    atts = [sb(f"atts{b}", [32, 32]) for b in range(2)]
    attT = [sb(f"attT{b}", [32, 32]) for b in range(2)]
    outs = sb("outs", [S, V])
    zb = sb("zb", [S, 1])

    # one PSUM bank per concurrently-live matmul output, reused over time
    B = [nc.alloc_psum_tensor(f"B{i}", [128, 128], f32) for i in range(6)]
    embp = B[0][0:D, 0:S]
    resp = [B[0][0:DA, 0:S], B[0][0:DA, 0:S]]
    qkp = [B[1][0:DA + _KOFF, 0:S], B[1][0:DA + _KOFF, 0:S]]
    vp = [B[2][0:S, 0:DA], B[2][0:S, 0:DA]]
    attp = [B[3][0:S, 0:S], B[4][0:S, 0:S]]
    hp = [B[5][0:D, 0:S], B[5][0:D, 0:S]]
    outp = B[5][0:S, 0:V]

    sD = nc.alloc_semaphore("sD")   # hot-half input DMA
    sE = nc.alloc_semaphore("sE")   # cold-half input DMA
    sP = nc.alloc_semaphore("sP")
    sA = nc.alloc_semaphore("sA")
    sV = nc.alloc_semaphore("sV")
    sO = nc.alloc_semaphore("sO")

    mask = C[0:S, _C_MSK:_C_MSK + S]

    # ---- SYNC: input DMAs first, output DMAs gated on DVE progress ----
    nc.sync.dma_start(C[:, 0:_C_HOT], consts[:, 0:_C_HOT]).then_inc(sD, 16)
    nc.sync.dma_start(C[:, _C_HOT:WCOL], consts[:, _C_HOT:WCOL]).then_inc(sE, 16)
    nc.sync.wait_ge(sV, 22)
    nc.sync.dma_start(att_d[:, :], atts[1][0:S, 0:S]).then_inc(sO, 16)
    nc.sync.wait_ge(sV, 26)
    nc.sync.dma_start(out_d[:, :], outs[:, :]).then_inc(sO, 16)
    if wait_out:
        nc.sync.wait_ge(sO, 32)

    # ---- ACT: k copies (moving operand of QK^T) and exps  (A1..A4) ----
    nc.scalar.wait_ge(sP, 2)
    nc.scalar.copy(ks[0][:, :], qkp[0][_KOFF:_KOFF + DA, :]).then_inc(sA)
    nc.scalar.wait_ge(sP, 5)
    nc.scalar.wait_ge(sV, 1)
    nc.scalar.activation(es[0][:, :], attp[0], EXP,
                         bias=zb[:, :]).then_inc(sA)                # A2
    nc.scalar.wait_ge(sP, 8)
    nc.scalar.copy(ks[1][:, :], qkp[1][_KOFF:_KOFF + DA, :]).then_inc(sA)
    nc.scalar.wait_ge(sP, 11)
    nc.scalar.activation(es[1][:, :], attp[1], EXP,
                         bias=zb[:, :]).then_inc(sA)                # A4

    # ---- DVE: memsets, adds, copies, softmax tail  (V1..V26) ----
    # engine writes are posted, so same-engine data deps also get sem edges
    nc.vector.memset(zb[:, :], 0.0).then_inc(sV)                    # V1
    for t, val in ((embA, 1.0), (h1A, 1.0), (resA[0], 1.0), (atts[0], 0.0),
                   (h2A, 1.0), (resA[1], 1.0), (atts[1], 0.0)):
        nc.vector.memset(t[:, :], val).then_inc(sV)                 # V2..V8
    nc.vector.wait_ge(sP, 1)
    nc.vector.wait_ge(sV, 2)
    nc.vector.tensor_add(embA[0:D, :], embp,
                         C[0:D, _C_POS:_C_POS + S]).then_inc(sV)    # V9
    for b, (pqk, pv, pres, ph, hnxt) in enumerate(
            ((2, 4, 6, 7, h1A), (8, 10, 12, 13, h2A))):
        v0 = 8 * b  # V10..V17 for b=0, V18..V25 for b=1
        nc.vector.wait_ge(sP, pqk)
        nc.vector.tensor_copy(qs[b][:, :], qkp[b][0:DA, :]).then_inc(sV)
        nc.vector.wait_ge(sP, pv)
        nc.vector.tensor_copy(vs[b][:, :], vp[b]).then_inc(sV)      # V11/ V19
        nc.vector.wait_ge(sA, 2 if b == 0 else 4)
        nc.vector.reduce_sum(ssum[b][:, :], es[b][:, :],
                             axis=X).then_inc(sV)                   # V12/ V20
        nc.vector.wait_ge(sV, v0 + 12)
        nc.vector.reciprocal(rr[b][:, :], ssum[b][:, :]).then_inc(sV)
        nc.vector.wait_ge(sV, v0 + 13)
        nc.vector.tensor_scalar_mul(atts[b][0:S, 0:S], es[b][:, :],
                                    rr[b][:, :]).then_inc(sV)       # V14/ V22
        nc.vector.wait_ge(sV, v0 + 14)
        nc.vector.transpose(attT[b][:, :], atts[b][:, :]).then_inc(sV)
        nc.vector.wait_ge(sP, pres)
        nc.vector.tensor_copy(resA[b][0:DA, :], resp[b]).then_inc(sV)
        nc.vector.wait_ge(sP, ph)
        nc.vector.tensor_copy(hnxt[0:D, :], hp[b]).then_inc(sV)     # V17/ V25
    nc.vector.wait_ge(sP, 14)
    nc.vector.tensor_copy(outs[:, :], outp).then_inc(sV)            # V26

    # ---- PE: the matmul chain  (P1..P14) ----
    # mask preloads are PE matmuls (id.T @ mask, start=True) so PSUM
    # has_written is set deterministically; an engine-write preload would
    # be OVERWRITTEN (not accumulated) by the QK^T matmul on a fresh bank.
    mm = nc.tensor.matmul
    idm = C[0:S, _C_ID:_C_ID + S]
    nc.tensor.wait_ge(sD, 16)
    mm(embp, C[0:29, _C_EMB:_C_EMB + D],
       C[0:29, _C_OH:_C_OH + S]).then_inc(sP)                       # P1
    # moving-operand waits are attached to the MM itself so the (const)
    # LDWEIGHTS prefetches during the stall; a pre-satisfied standalone
    # wait before the next matmul stops LDW pull-ahead past a stalled MM
    # wherever that LDW reads guarded data.
    for b, (qkoff, voff, foff, hA, vw, aw, kw) in enumerate(
            ((_C_QK0, _C_V0, _C_F0, embA, 9, 1, 10),
             (_C_QK1, _C_V1, _C_F1, h1A, 17, 3, 18))):
        mm(qkp[b], C[0:8, qkoff:qkoff + _KOFF + DA],
           hA[:, :])._wait_ge(sV, vw).then_inc(sP)                  # P2 / P8
        if b == 0:
            nc.tensor.wait_ge(sE, 16)
        mm(attp[b], idm, mask).then_inc(sP)                         # P3 / P9
        nc.tensor.wait_ge(sV, vw)   # anti-pull-ahead: v's LDW reads hA
        mm(vp[b], hA[:, :], C[0:8, voff:voff + DA]).then_inc(sP)    # P4 / P10
        nc.tensor.wait_ge(sV, kw)   # qs ready (stationary of QK^T)
        mm(attp[b], qs[b][:, :], ks[b][:, :], start=False, stop=True,
           skip_group_check=True)._wait_ge(sA, aw).then_inc(sP)     # P5 / P11
        nc.tensor.wait_ge(sV, 11 if b == 0 else 19)  # vs ready for LDW
        mm(resp[b], vs[b][:, :],
           attT[b][0:S, 0:S])._wait_ge(sV, 15 if b == 0 else 23).then_inc(sP)
        mm(hp[b], C[0:12, foff:foff + D],
           resA[b][:, :])._wait_ge(sV, 16 if b == 0 else 24).then_inc(sP)
    nc.tensor.wait_ge(sV, 25)
    mm(outp, h2A[:, :], C[0:8, _C_OUT:_C_OUT + V]).then_inc(sP)     # P14

    nc.compile()
    return nc


def _get_nc():
    if "nc" not in _cache:
        _cache["nc"] = build(wait_out=False)
    return _cache["nc"]


def pack_consts(inputs):
    x = np.asarray(inputs["x"]).astype(np.int64)
    C = np.zeros((29, WCOL), np.float32)
    C[:, _C_EMB:_C_EMB + D] = np.asarray(inputs["emb_table"], np.float32)
    C[x, _C_OH + np.arange(S)] = 1.0
    for off, (wq, bq, wk, bk) in (
        (_C_QK0, ("w_q0", "b_q0", "w_k0", "b_k0")),
        (_C_QK1, ("w_q1", "b_q1", "w_k1", "b_k1")),
    ):
        C[0:D, off:off + DA] = np.asarray(inputs[wq], np.float32).T
        C[D, off:off + DA] = np.asarray(inputs[bq], np.float32)
        C[0:D, off + _KOFF:off + _KOFF + DA] = np.asarray(inputs[wk], np.float32).T
        C[D, off + _KOFF:off + _KOFF + DA] = np.asarray(inputs[bk], np.float32)
    for off, (wv, bv) in ((_C_V0, ("w_v0", "b_v0")), (_C_V1, ("w_v1", "b_v1"))):
        C[0:D, off:off + DA] = np.asarray(inputs[wv], np.float32).T
        C[D, off:off + DA] = np.asarray(inputs[bv], np.float32)
    for off, (wf, bf) in ((_C_F0, ("w_f0", "b_f0")), (_C_F1, ("w_f1", "b_f1"))):
        C[0:DA, off:off + D] = np.asarray(inputs[wf], np.float32).T
        C[DA, off:off + D] = np.asarray(inputs[bf], np.float32)
    C[0:D, _C_OUT:_C_OUT + V] = np.asarray(inputs["w_out"], np.float32).T
    C[D, _C_OUT:_C_OUT + V] = np.asarray(inputs["b_out"], np.float32)
    C[0:D, _C_POS:_C_POS + S] = np.asarray(inputs["pos"], np.float32)[:S].T
    C[0:S, _C_MSK:_C_MSK + S] = np.where(
        np.tril(np.ones((S, S), bool)), 0.0, NEG).astype(np.float32)
    C[0:S, _C_ID:_C_ID + S] = np.eye(S, dtype=np.float32)
    return C


def run(inputs, trace=False):
    """Run on HW; returns (results_dict_core0, BassKernelResults)."""
    from concourse.bass_utils import run_bass_kernel_spmd

    nc = _get_nc()
    in_map = {"consts": pack_consts(inputs)}
    res = run_bass_kernel_spmd(
        nc, [dict(in_map) for _ in range(N_CORES)], list(range(N_CORES)),
        trace=trace,
    )
    return res.results[0], res


def kernel(**inputs):
    r0, _ = run(inputs, trace=False)
    return r0["out"].astype(np.float32), r0["att"].astype(np.float32)
